# revision 9
# baseline (speedup 1.0000x reference)
"""EngramEmbeddings Trainium2 kernel.

Expert-sharded across 8 NeuronCores: core c owns head c of the n=2 and n=3
hash tables and computes the hashed-ngram embedding lookup for all
B*S = 32768 tokens for its two slots.

The previous design was bound by GPSIMD (Q7) descriptor generation: every
gathered row costs ~8.8ns of descriptor-emit time on ONE Q7 core pair, and
both the dma_gather (n2) and indirect-DMA (n3) paths serialized on pairs
(2,3)-(6,7) and (0,1) respectively, ~575us/core.  This version spreads ALL
gathers across the four SWDGE queues: queue q's descriptors are generated
by Q7 core pair (2q, 2q+1), and queues 1-3 retire instantly on the engine
(cores 0/1 are idle for them) so their pairs work concurrently; queue 0's
instructions (whose worker pair is the engine's completion pair) are issued
last in each round so they overlap the others.  ~4x more Q7 throughput.

n3 tables exceed dma_gather's int16 index reach (65579 rows > 32768), so
rows are packed 3-per-512B-super-row in bf16 (3 x 160B + 32B pad); the
device gathers super = idx//3 and selects the 80-wide sub-row with two
arithmetic-select steps on the vector engine (masks broadcast via 0-step
APs).  n2 rows are bf16 padded to 256B and gathered directly.  Tables are
bf16 (norm rel err ~2^-9, tolerance is 2e-2); outputs are written bf16 and
cast to f32 on the host.

The int64 hash is computed exactly with 16-bit limb arithmetic on the
vector engine (as before), but only THREE product streams (cur*s0, cur*s1,
cur*s2) are computed on a 258-column extended id tile; the prev/prev2
operands are just shifted column slices of the same product limbs.
"""

import numpy as np

try:
    import concourse  # noqa: F401
except ImportError:  # pragma: no cover
    import sys

    for _p in ("/opt/trn_rl_repo", "/root/.axon_site/_ro/trn_rl_repo"):
        if _p not in sys.path:
            sys.path.insert(0, _p)

import ml_dtypes

import concourse.tile as tile
from concourse import bacc, mybir
from concourse.bass_utils import run_bass_kernel_spmd

N2_SIZES = [6619, 6637, 6653, 6659, 6661, 6673, 6679, 6689]
N3_SIZES = [65521, 65537, 65539, 65543, 65551, 65557, 65563, 65579]
B, S = 8, 4096
P = 128
NTOK = B * S               # 32768
TPB = NTOK // P            # 256 token cols (p-major: token = p*256 + c)
SLOT = 80
V2 = max(N2_SIZES)         # 6689 rows, bf16 row padded to 128 elems = 256B
S3 = -(-max(N3_SIZES) // 3)  # 21860 super rows of 3 packed bf16 rows (512B)
E2 = 128                   # n2 bf16 elems per row (256B)
E3 = 256                   # n3 bf16 elems per super row (512B)
CHT = 2048                 # tokens per gather chunk
CW = CHT // P              # 16 token cols per chunk
NCH = TPB // CW            # 16 chunks per slot kind
NQ = 4
RPQ = NCH // NQ            # 4 rounds per queue
XC = TPB + 2               # extended id cols

_NC = None
TRACE = False
LAST_RESULT = None


def _build_nc():
    dt = mybir.dt
    A = mybir.AluOpType
    AND, XOR = A.bitwise_and, A.bitwise_xor
    LSR, LSL = A.logical_shift_right, A.logical_shift_left
    ADD, MULT, SUB, GE = A.add, A.mult, A.subtract, A.is_ge
    i32, i16 = dt.int32, dt.int16
    f32, bf16 = dt.float32, dt.bfloat16

    nc = bacc.Bacc("TRN2", target_bir_lowering=False, debug=False,
                   num_swdge_queues=4)
    tbl2 = nc.dram_tensor("tbl2", [V2, E2], bf16, kind="ExternalInput")
    tbl3 = nc.dram_tensor("tbl3", [S3, E3], bf16, kind="ExternalInput")
    idsd = nc.dram_tensor("ids", [P, XC], i32, kind="ExternalInput")
    s0d = nc.dram_tensor("s0w", [P, 3 * XC], i32, kind="ExternalInput")
    s1d = nc.dram_tensor("s1w", [P, 3 * XC], i32, kind="ExternalInput")
    # per-slot mod constants: M, R16, R24, R32, R40 (int32) + inv (f32)
    cst3d = nc.dram_tensor("cst3", [P, 5 * TPB], i32, kind="ExternalInput")
    cst2d = nc.dram_tensor("cst2", [P, 5 * TPB], i32, kind="ExternalInput")
    inv3d = nc.dram_tensor("inv3", [P, TPB], f32, kind="ExternalInput")
    inv2d = nc.dram_tensor("inv2", [P, TPB], f32, kind="ExternalInput")
    out2d = nc.dram_tensor("out2", [NTOK, SLOT], bf16, kind="ExternalOutput")
    out3d = nc.dram_tensor("out3", [NTOK, SLOT], bf16, kind="ExternalOutput")

    with tile.TileContext(nc) as tc:
        with (
            tc.tile_pool(name="c", bufs=1) as cp,
            tc.tile_pool(name="w", bufs=1) as wp,
            tc.tile_pool(name="g", bufs=1) as gp,
        ):

            def ld(dram, shape, dtype, tag):
                t = cp.tile(shape, dtype, tag=tag, name=tag)
                nc.sync.dma_start(t[:], dram.ap())
                return t

            ids = ld(idsd, [P, XC], i32, "ids")
            s0w = ld(s0d, [P, 3 * XC], i32, "s0w")
            s1w = ld(s1d, [P, 3 * XC], i32, "s1w")
            cst3 = ld(cst3d, [P, 5 * TPB], i32, "cst3")
            cst2 = ld(cst2d, [P, 5 * TPB], i32, "cst2")
            inv3 = ld(inv3d, [P, TPB], f32, "inv3")
            inv2 = ld(inv2d, [P, TPB], f32, "inv2")

            idx2_16 = cp.tile([P, TPB], i16, tag="idx2_16", name="idx2_16")
            sup3_16 = cp.tile([P, TPB], i16, tag="sup3_16", name="sup3_16")
            # bf16 sub-row select masks for n3: m1 = (s>=1), m2 = (s>=2)
            m1t = cp.tile([P, TPB], bf16, tag="m1", name="m1")
            m2t = cp.tile([P, TPB], bf16, tag="m2", name="m2")

            # ---- hash pipeline, one slab of SW cols at a time ----
            SW = 64          # token cols per slab
            SX = SW + 2      # extended cols needed for the slab's products

            def wt(w):
                return wp.tile([P, w], i32, tag=f"w{w}", bufs=12,
                               name=f"w_{nc.next_id()}")

            def lt(j, l):
                # product limb tiles live across the whole slab
                return wp.tile([P, SX], i32, tag=f"L{j}{l}", bufs=2,
                               name=f"L{j}{l}_{nc.next_id()}")

            def st(dtype=i32):
                return wp.tile([P, SW], dtype, tag=f"s{dtype}", bufs=40,
                               name=f"s_{nc.next_id()}")

            def product_limbs(c0):
                """Limbs (L0, L1, L2) of ids_ext * seed_j on cols
                [c0, c0+SX) for j = 0,1,2, all int32 [P, SX]."""
                x = ids[:, c0 : c0 + SX]
                a0 = wt(SX)
                nc.vector.tensor_scalar(a0[:], x, 0xFF, None, AND)
                a1 = wt(SX)
                nc.vector.tensor_scalar(a1[:], x, 8, None, LSR)
                out = []
                for j in range(3):
                    sl = s0w[:, j * XC + c0 : j * XC + c0 + SX]
                    sh = s1w[:, j * XC + c0 : j * XC + c0 + SX]
                    t00, t10, t01, t11 = wt(SX), wt(SX), wt(SX), wt(SX)
                    nc.vector.tensor_tensor(t00[:], a0[:], sl, MULT)
                    nc.vector.tensor_tensor(t10[:], a1[:], sl, MULT)
                    nc.vector.tensor_tensor(t01[:], a0[:], sh, MULT)
                    nc.vector.tensor_tensor(t11[:], a1[:], sh, MULT)
                    Apt = wt(SX)
                    nc.vector.tensor_scalar(Apt[:], t10[:], 0xFF, 8, AND, LSL)
                    v0 = wt(SX)
                    nc.vector.tensor_scalar(v0[:], t00[:], 0xFFFF, None, AND)
                    nc.vector.tensor_tensor(v0[:], v0[:], Apt[:], ADD)
                    L0 = lt(j, 0)
                    nc.vector.tensor_scalar(L0[:], v0[:], 0xFFFF, None, AND)
                    c0_ = wt(SX)
                    nc.vector.tensor_scalar(c0_[:], v0[:], 16, None, LSR)
                    u1 = wt(SX)
                    nc.vector.tensor_scalar(u1[:], t10[:], 8, None, LSR)
                    nc.vector.tensor_tensor(u1[:], u1[:], c0_[:], ADD)
                    u2 = wt(SX)
                    nc.vector.tensor_scalar(u2[:], t01[:], 0xFFFF, None, AND)
                    nc.vector.tensor_tensor(u2[:], u2[:], u1[:], ADD)
                    v1 = wt(SX)
                    nc.vector.tensor_scalar(v1[:], t00[:], 16, None, LSR)
                    nc.vector.tensor_tensor(v1[:], v1[:], u2[:], ADD)
                    Ff = wt(SX)
                    nc.vector.tensor_scalar(Ff[:], t11[:], 0xFF, 8, AND, LSL)
                    nc.vector.tensor_tensor(v1[:], v1[:], Ff[:], ADD)
                    L1 = lt(j, 1)
                    nc.vector.tensor_scalar(L1[:], v1[:], 0xFFFF, None, AND)
                    c1 = wt(SX)
                    nc.vector.tensor_scalar(c1[:], v1[:], 16, None, LSR)
                    v2 = wt(SX)
                    nc.vector.tensor_scalar(v2[:], t01[:], 16, None, LSR)
                    nc.vector.tensor_tensor(v2[:], v2[:], c1[:], ADD)
                    L2 = lt(j, 2)
                    nc.vector.tensor_scalar(L2[:], t11[:], 8, None, LSR)
                    nc.vector.tensor_tensor(L2[:], L2[:], v2[:], ADD)
                    out.append((L0, L1, L2))
                return out

            def mod_m(x, Mt, INV, correct):
                """x mod m; exact in [0, 2m) (correct=False) or [0, m)."""
                y = st(f32)
                nc.vector.tensor_tensor(y[:], x[:], INV, MULT)
                nc.vector.tensor_scalar(y[:], y[:], 0.5, None, SUB)
                q = st()
                nc.vector.tensor_copy(q[:], y[:])
                qm = st()
                nc.vector.tensor_tensor(qm[:], q[:], Mt, MULT)
                r = st()
                nc.vector.tensor_tensor(r[:], x[:], qm[:], SUB)
                if not correct:
                    return r
                ge = st()
                nc.vector.tensor_tensor(ge[:], r[:], Mt, GE)
                gm = st()
                nc.vector.tensor_tensor(gm[:], ge[:], Mt, MULT)
                nc.vector.tensor_tensor(r[:], r[:], gm[:], SUB)
                return r

            def slab_idx(limbs, shifts, cst, inv, c0):
                """Table index [P, SW] int32 in [0, m) for one slot.

                limbs: list of (L0, L1, L2) per term j; shifts[j] gives the
                column shift (2 - ngram distance) into the SX-wide tiles.
                """
                cs = slice(c0, c0 + SW)
                Mt = cst[:, 0 * TPB + c0 : 0 * TPB + c0 + SW]
                R16 = cst[:, 1 * TPB + c0 : 1 * TPB + c0 + SW]
                R24 = cst[:, 2 * TPB + c0 : 2 * TPB + c0 + SW]
                R32 = cst[:, 3 * TPB + c0 : 3 * TPB + c0 + SW]
                R40 = cst[:, 4 * TPB + c0 : 4 * TPB + c0 + SW]
                INV = inv[:, cs]
                H = []
                for l in range(3):
                    Ht = st()
                    nc.vector.tensor_tensor(
                        Ht[:], limbs[0][l][:, shifts[0] : shifts[0] + SW],
                        limbs[1][l][:, shifts[1] : shifts[1] + SW], XOR)
                    for j in range(2, len(limbs)):
                        nc.vector.tensor_tensor(
                            Ht[:], Ht[:],
                            limbs[j][l][:, shifts[j] : shifts[j] + SW], XOR)
                    H.append(Ht)
                H0, H1, H2 = H
                H1a = st()
                nc.vector.tensor_scalar(H1a[:], H1[:], 0xFF, None, AND)
                H1b = st()
                nc.vector.tensor_scalar(H1b[:], H1[:], 8, None, LSR)
                H2a = st()
                nc.vector.tensor_scalar(H2a[:], H2[:], 0xFF, None, AND)
                H2b = st()
                nc.vector.tensor_scalar(H2b[:], H2[:], 8, None, LSR)
                ps = []
                for piece, R in ((H1a, R16), (H1b, R24), (H2a, R32),
                                 (H2b, R40)):
                    pp = st()
                    nc.vector.tensor_tensor(pp[:], piece[:], R, MULT)
                    ps.append(mod_m(pp, Mt, INV, correct=False))
                x1 = st()
                nc.vector.tensor_tensor(x1[:], H0[:], ps[0][:], ADD)
                x2 = st()
                nc.vector.tensor_tensor(x2[:], ps[1][:], ps[2][:], ADD)
                nc.vector.tensor_tensor(x1[:], x1[:], x2[:], ADD)
                nc.vector.tensor_tensor(x1[:], x1[:], ps[3][:], ADD)
                return mod_m(x1, Mt, INV, correct=True)

            def hash_slab(sb):
                """Hash token cols [64*sb, 64*sb+64): fill idx2_16, sup3_16,
                m1t, m2t."""
                c0 = SW * sb
                limbs = product_limbs(c0)
                cs = slice(c0, c0 + SW)
                # n2: h = prv*s0 ^ cur*s1 -> shifts (1, 2) into limbs 0, 1
                idx2 = slab_idx([limbs[0], limbs[1]], [1, 2], cst2, inv2, c0)
                nc.vector.tensor_copy(idx2_16[:, cs], idx2[:])
                # n3: h = pv2*s0 ^ prv*s1 ^ cur*s2 -> shifts (0, 1, 2)
                idx3 = slab_idx(limbs, [0, 1, 2], cst3, inv3, c0)
                # super = idx3 // 3 exactly; s = idx3 - 3*super in {0,1,2}
                y = st(f32)
                nc.vector.tensor_scalar(y[:], idx3[:], (1.0 / 3.0) * (1 - 1e-6),
                                        0.5, MULT, SUB)
                q = st()
                nc.vector.tensor_copy(q[:], y[:])
                r = st()
                nc.vector.tensor_scalar(r[:], q[:], 3, None, MULT)
                nc.vector.tensor_tensor(r[:], idx3[:], r[:], SUB)
                ge = st()
                nc.vector.tensor_scalar(ge[:], r[:], 3, None, GE)
                nc.vector.tensor_tensor(q[:], q[:], ge[:], ADD)
                g3 = st()
                nc.vector.tensor_scalar(g3[:], ge[:], 3, None, MULT)
                nc.vector.tensor_tensor(r[:], r[:], g3[:], SUB)
                nc.vector.tensor_copy(sup3_16[:, cs], q[:])
                nc.vector.tensor_scalar(m1t[:, cs], r[:], 1, None, GE)
                nc.vector.tensor_scalar(m2t[:, cs], r[:], 2, None, GE)

            # ---- idx transport: p-major idx -> per-queue wrapped stream ----
            # stream pos j of queue q's round-k chunk: lane q' = j%16,
            # sc = j//16; dest (partition u = 16*(sc%8)+q', block b = sc//8).
            # chunk ck = 4k+q covers p-major cols [16ck, 16ck+16), token at
            # (u, b) = col 16ck+b.  So stg[32q+16h+q', 128k+8b+c8] =
            # idx[16c8+q', 64k+16q+b].
            stg2 = [cp.tile([P, RPQ * P], i16, tag=f"stg2_{q}",
                            name=f"stg2_{q}") for q in range(NQ)]
            stg3 = [cp.tile([P, RPQ * P], i16, tag=f"stg3_{q}",
                            name=f"stg3_{q}") for q in range(NQ)]

            def transport(src16, stg, q, rk):
                """Move queue q's round-rk chunk idx into its stg tile."""
                for c8 in range(8):
                    s_ap = src16[16 * c8 : 16 * c8 + 16,
                                 64 * rk + 16 * q : 64 * rk + 16 * q + 16]
                    for h in range(2):
                        d_ap = stg[q][
                            32 * q + 16 * h : 32 * q + 16 * h + 16, :
                        ].rearrange("p (k b e) -> p k b e", k=RPQ, b=CW,
                                    e=8)[:, rk, :, c8]
                        nc.scalar.dma_start(d_ap, s_ap)

            # ---- gathers ----
            out2v = out2d.ap().rearrange("(p t) d -> p t d", p=P)
            out3v = out3d.ap().rearrange("(p t) d -> p t d", p=P)

            def n2_chunk(q, rk):
                ck = 4 * rk + q
                d2 = gp.tile([P, CW * E2], bf16, tag=f"d2_{q}", bufs=2,
                             name=f"d2_{q}_{rk}")
                nc.gpsimd.dma_gather(
                    d2[:].rearrange("p (b e) -> p b e", e=E2),
                    tbl2.ap(),
                    stg2[q][:, P * rk : P * rk + P],
                    CHT,
                    CHT,
                    E2,
                    single_packet=False,
                    queue_num=q,
                )
                nc.sync.dma_start(
                    out2v[:, CW * ck : CW * (ck + 1), :],
                    d2[:].rearrange("p (b e) -> p b e", e=E2)[:, :, :SLOT],
                )

            def n3_chunk(q, rk):
                ck = 4 * rk + q
                cs = slice(CW * ck, CW * (ck + 1))
                d3 = gp.tile([P, CW * E3], bf16, tag=f"d3_{q}", bufs=2,
                             name=f"d3_{q}_{rk}")
                nc.gpsimd.dma_gather(
                    d3[:].rearrange("p (b e) -> p b e", e=E3),
                    tbl3.ap(),
                    stg3[q][:, P * rk : P * rk + P],
                    CHT,
                    CHT,
                    E3,
                    single_packet=False,
                    queue_num=q,
                )
                # arithmetic sub-row select: out = g0 + m1*(g1-g0) + m2*(g2-g1)
                gv = d3[:].rearrange("p (b e) -> p b e", e=E3)
                g0 = gv[:, :, 0 * SLOT : 1 * SLOT]
                g1 = gv[:, :, 1 * SLOT : 2 * SLOT]
                g2 = gv[:, :, 2 * SLOT : 3 * SLOT]

                def bcast(m):
                    return m[:, cs].rearrange("p (b one) -> p b one",
                                              one=1).to_broadcast(
                                                  [P, CW, SLOT])

                d10 = gp.tile([P, CW * SLOT], bf16, tag=f"sel_{q}", bufs=2,
                              name=f"d10_{q}_{rk}")
                dv = d10[:].rearrange("p (b e) -> p b e", e=SLOT)
                acc = gp.tile([P, CW * SLOT], bf16, tag=f"acc_{q}", bufs=2,
                              name=f"acc_{q}_{rk}")
                av = acc[:].rearrange("p (b e) -> p b e", e=SLOT)
                nc.vector.tensor_tensor(dv, g1, g0, SUB)
                nc.vector.tensor_tensor(dv, dv, bcast(m1t), MULT)
                nc.vector.tensor_tensor(av, g0, dv, ADD)
                d21 = gp.tile([P, CW * SLOT], bf16, tag=f"sel_{q}", bufs=2,
                              name=f"d21_{q}_{rk}")
                ev = d21[:].rearrange("p (b e) -> p b e", e=SLOT)
                nc.vector.tensor_tensor(ev, g2, g1, SUB)
                nc.vector.tensor_tensor(ev, ev, bcast(m2t), MULT)
                nc.vector.tensor_tensor(av, av, ev, ADD)
                nc.sync.dma_start(out3v[:, cs, :], av)

            # ---- schedule ----
            # slab s feeds round s (n2) and round s (n3).  Emit n2 round r
            # and n3 round r back to back so both kinds pipeline across the
            # four queues; queue 0 last (it blocks the engine).
            for sb in range(4):
                hash_slab(sb)
                for q in (1, 2, 3, 0):
                    transport(idx2_16, stg2, q, sb)
                    transport(sup3_16, stg3, q, sb)
            for rk in range(RPQ):
                for q in (1, 2, 3, 0):
                    n2_chunk(q, rk)
                for q in (1, 2, 3, 0):
                    n3_chunk(q, rk)

    nc.compile()
    return nc


def _get_nc():
    global _NC
    if _NC is None:
        _NC = _build_nc()
    return _NC


def _broadcast_rows(row):
    return np.ascontiguousarray(np.broadcast_to(row, (P, row.shape[0])))


def _mod_consts(m):
    return [m, 2**16 % m, 2**24 % m, 2**32 % m, 2**40 % m]


def _make_in_maps(inputs):
    ids = np.asarray(inputs["canonical_ids"]).astype(np.int64)  # [B, S]
    hs = np.asarray(inputs["hash_seeds"]).astype(np.int64)      # [3, 8]
    cur = ids.reshape(-1).astype(np.int32)                       # p-major
    ext = np.zeros((P, XC), np.int32)
    ext[:, 2:] = cur.reshape(P, TPB)
    t0 = np.arange(P) * TPB
    inner = t0 % S != 0  # partitions whose first token is not a batch start
    ext[inner, 1] = cur[t0[inner] - 1]
    ext[inner, 0] = cur[t0[inner] - 2]

    maps = []
    for c in range(8):
        s0, s1, s2 = int(hs[0, c]), int(hs[1, c]), int(hs[2, c])
        m2, m3 = N2_SIZES[c], N3_SIZES[c]

        def seed_row(lo):
            return np.concatenate(
                [np.full(XC, (sd & 0xFFFF) if lo else (sd >> 16), np.int32)
                 for sd in (s0, s1, s2)]
            )

        cst3row = np.concatenate(
            [np.full(TPB, v, np.int32) for v in _mod_consts(m3)]
        )
        cst2row = np.concatenate(
            [np.full(TPB, v, np.int32) for v in _mod_consts(m2)]
        )
        inv3row = np.full(TPB, np.float64(1.0 / m3) * (1 - 1e-6), np.float32)
        inv2row = np.full(TPB, np.float64(1.0 / m2) * (1 - 1e-6), np.float32)

        w2 = np.asarray(inputs[f"w_n2_h{c}"], dtype=np.float32)
        tbl2 = np.zeros((V2, E2), ml_dtypes.bfloat16)
        tbl2[: w2.shape[0], :SLOT] = w2.astype(ml_dtypes.bfloat16)

        w3 = np.asarray(inputs[f"w_n3_h{c}"], dtype=np.float32)
        w3p = np.zeros((3 * S3, SLOT), ml_dtypes.bfloat16)
        w3p[: w3.shape[0]] = w3.astype(ml_dtypes.bfloat16)
        tbl3 = np.zeros((S3, E3), ml_dtypes.bfloat16)
        tbl3[:, : 3 * SLOT] = w3p.reshape(S3, 3 * SLOT)

        maps.append(
            {
                "tbl2": tbl2,
                "tbl3": tbl3,
                "ids": ext,
                "s0w": _broadcast_rows(seed_row(True)),
                "s1w": _broadcast_rows(seed_row(False)),
                "cst3": _broadcast_rows(cst3row),
                "cst2": _broadcast_rows(cst2row),
                "inv3": _broadcast_rows(inv3row),
                "inv2": _broadcast_rows(inv2row),
            }
        )
    return maps


def kernel(**inputs):
    global LAST_RESULT
    nc = _get_nc()
    in_maps = _make_in_maps(inputs)
    res = run_bass_kernel_spmd(nc, in_maps, core_ids=list(range(8)),
                               trace=TRACE)
    LAST_RESULT = res
    out = np.empty((B, S, 16 * SLOT), np.float32)
    for c in range(8):
        o2 = res.results[c]["out2"].astype(np.float32).reshape(B, S, SLOT)
        o3 = res.results[c]["out3"].astype(np.float32).reshape(B, S, SLOT)
        out[:, :, c * SLOT : (c + 1) * SLOT] = o2
        out[:, :, (8 + c) * SLOT : (9 + c) * SLOT] = o3
    return out
